# revision 16
# baseline (speedup 1.0000x reference)
"""Grouped MoE MLP (SwiGLU) for TRN2, expert-parallel across 8 NeuronCores.

Problem: T=8192 tokens pre-permuted into 8 contiguous expert segments of 1024,
H=1024, I=2816, per-expert weights gate/up [H,I], down [I,H].
    o1 = x @ gate; o2 = x @ up; h = silu(o1)*o2; out = h @ down

Sharding: expert-parallel - core e computes expert e's segment entirely
(zero collectives). Host slices inputs per expert and concatenates outputs.

Device kernel (per core), all matmul inputs bf16 (PSUM accumulation fp32,
rel err ~4e-3), N=512 moving dim (one PSUM bank):
  - stage 1: per I-slab of 128: o1T/o2T [128i, TE] = gate/up-slab.T @ xt,
    PSUM-accumulated over 8 H-chunks; SwiGLU fused: hT = silu(o1T)*o2T
    kept resident in SBUF ([I, TE] bf16, 22 slabs).
  - stage 2: out[TE, H] = hT.T @ down, PSUM-accumulated over 22 I-slabs.

Perf structure:
  - ~20 dependency-free warmup matmuls on scratch SBUF keep the PE busy
    from t~0 so the HAM clock-gate reaches 8/8 before real work and the
    DMA wait for the first tiles is hidden.
  - All DRAM layouts give every DMA >=2KB contiguous per-partition rows
    (4KB mostly): xt h-chunk pairs, gate+up packed per slab, down slab
    pairs. Each dma_start fans out across 16 HW queues.
  - dma_start issues cost ~0.6us of issuing-engine time each, so loads
    are spread round-robin over the Sync/GpSimd/Vector queues instead of
    serializing on one engine; outputs go on Scalar.
"""

import numpy as np
from contextlib import ExitStack

E, H, I, T = 8, 1024, 2816, 8192
TE = T // E  # tokens per expert = 1024
KC = H // 128  # 8 h-chunks
IS = I // 128  # 22 i-slabs
NCH = 512  # moving free dim per matmul (one PSUM bank of fp32)
NXP = KC // 2  # 4 xt pair tiles
NDP = IS // 2  # 11 down pair tiles
N_WARMUP = 22

_cache: dict = {}


def _build_nc():
    from concourse import bacc
    import concourse.tile as tile
    import concourse.mybir as mybir

    f32 = mybir.dt.float32
    bf = mybir.dt.bfloat16

    nc = bacc.Bacc("TRN2", target_bir_lowering=False, debug=False, num_devices=8)
    # xt pair j holds h-chunks k=2j,2j+1; quarter q = (k%2)*2 + tc
    xt_d = nc.dram_tensor("xt", [NXP, 128, 4, NCH], bf, kind="ExternalInput").ap()
    # gate+up packed per slab: slot k = gate h-chunk k, slot KC+k = up
    gu_d = nc.dram_tensor("gu", [IS, 128, 2 * KC, 128], bf, kind="ExternalInput").ap()
    # down pair j holds i-slabs 2j,2j+1; quarter q = (i%2)*2 + hc
    dn_d = nc.dram_tensor("dn", [NDP, 128, 4, NCH], bf, kind="ExternalInput").ap()
    out_d = nc.dram_tensor("out", [TE, H], f32, kind="ExternalOutput").ap()

    silu_fn = mybir.ActivationFunctionType.Silu

    with tile.TileContext(nc) as tc, ExitStack() as ctx:
        xt_pool = ctx.enter_context(tc.tile_pool(name="xt", bufs=NXP))
        gu_pool = ctx.enter_context(tc.tile_pool(name="gu", bufs=4))
        h_pool = ctx.enter_context(tc.tile_pool(name="h", bufs=IS))
        d_pool = ctx.enter_context(tc.tile_pool(name="d", bufs=NDP))
        s_pool = ctx.enter_context(tc.tile_pool(name="s", bufs=2))
        o_pool = ctx.enter_context(tc.tile_pool(name="o", bufs=2))
        w_pool = ctx.enter_context(tc.tile_pool(name="w", bufs=1))
        ps1 = ctx.enter_context(tc.tile_pool(name="ps1", bufs=2, space="PSUM"))
        ps3 = ctx.enter_context(tc.tile_pool(name="ps3", bufs=2, space="PSUM"))
        psw = ctx.enter_context(tc.tile_pool(name="psw", bufs=1, space="PSUM"))

        # DGE priming: issue a tiny DMA on each HW-DGE engine first so the
        # descriptor-fetch/ring-start latency (~2.4us) is paid on these
        # instead of the first real loads.
        prime = w_pool.tile([128, 256], bf, tag="wp", name="prime")
        nc.sync.dma_start(out=prime[0:1, 0:128], in_=xt_d[0, 0:1, 0, 0:128])
        nc.scalar.dma_start(out=prime[0:1, 128:256], in_=xt_d[0, 0:1, 1, 0:128])

        # PE warmup: dependency-free matmuls on scratch data so the PE is
        # busy while the first input DMAs are in flight (HAM un-throttles
        # after ~3.4us of sustained activity).
        dmy = w_pool.tile([128, 256], bf, tag="w", name="dmy")
        nc.vector.memset(dmy[:], 0)
        pw = psw.tile([128, 256], f32, tag="pw", name="pw")
        for _ in range(N_WARMUP):
            nc.tensor.matmul(
                pw[:], lhsT=dmy[:, 0:128], rhs=dmy[:], start=True, stop=True
            )

        # wave 1: critical input loads. Only Sync and Scalar have hardware
        # DGE paths (GpSimd DMAs fall back to slow software DGE - never
        # use them for bulk data). Transfers complete in ISSUE order (all
        # dma_starts share the same 16 HW queues FIFO at ~300GB/s), so the
        # order below is the arrival order, matched to slab-0's k-order
        # consumption; everything is split in 256KB halves so supply
        # granularity (~0.85us) matches the 4-matmuls-per-k consumption
        # rate (~0.86us).
        xts = [xt_pool.tile([128, 4, NCH], bf, tag="xt", name=f"xt{j}")
               for j in range(NXP)]
        gus = {}
        for i in range(3):
            gus[i] = gu_pool.tile([128, 2 * KC, 128], bf, tag="gu", name=f"gu{i}")
        nc.sync.dma_start(out=gus[0][:, 0:KC, :], in_=gu_d[0, :, 0:KC])
        nc.scalar.dma_start(out=gus[0][:, KC : 2 * KC, :], in_=gu_d[0, :, KC : 2 * KC])
        for j in range(NXP):
            nc.sync.dma_start(out=xts[j][:, 0:2, :], in_=xt_d[j, :, 0:2])
            nc.scalar.dma_start(out=xts[j][:, 2:4, :], in_=xt_d[j, :, 2:4])
        nc.sync.dma_start(out=gus[1][:, 0:KC, :], in_=gu_d[1, :, 0:KC])
        nc.scalar.dma_start(out=gus[1][:, KC : 2 * KC, :], in_=gu_d[1, :, KC : 2 * KC])
        nc.sync.dma_start(out=gus[2][:, 0:KC, :], in_=gu_d[2, :, 0:KC])
        nc.scalar.dma_start(out=gus[2][:, KC : 2 * KC, :], in_=gu_d[2, :, KC : 2 * KC])

        def issue(dst, src):
            nc.sync.dma_start(out=dst, in_=src)

        # stage 1: per i-slab, o1T/o2T then fused SwiGLU into resident hT
        hts = []
        dns = {}
        for i in range(IS):
            if i + 3 < IS:
                gus[i + 3] = gu_pool.tile(
                    [128, 2 * KC, 128], bf, tag="gu", name=f"gu{i + 3}"
                )
                issue(gus[i + 3][:], gu_d[i + 3])
            if 0 <= i - 10 < NDP:
                j = i - 10
                dns[j] = d_pool.tile([128, 4, NCH], bf, tag="d", name=f"dn{j}")
                issue(dns[j][:], dn_d[j])
            g = gus[i]
            ht = h_pool.tile([128, TE], bf, tag="h")
            if i == 0:
                # slab 0 is DMA-bound: interleave k across all four PSUM
                # groups so the PE consumes each arriving xt tile at the
                # DMA supply rate instead of idling for the full set.
                p1s = [ps1.tile([128, NCH], f32, tag="p1", name=f"p1_{t}")
                       for t in range(2)]
                p2s = [ps1.tile([128, NCH], f32, tag="p2", name=f"p2_{t}")
                       for t in range(2)]
                for k in range(KC):
                    for tc_i in range(2):
                        rhs = xts[k // 2][:, (k % 2) * 2 + tc_i, :]
                        nc.tensor.matmul(
                            p1s[tc_i][:], lhsT=g[:, k, :], rhs=rhs,
                            start=(k == 0), stop=(k == KC - 1),
                        )
                        nc.tensor.matmul(
                            p2s[tc_i][:], lhsT=g[:, KC + k, :], rhs=rhs,
                            start=(k == 0), stop=(k == KC - 1),
                        )
                for tc_i in range(2):
                    sl = s_pool.tile([128, NCH], f32, tag="s")
                    nc.scalar.activation(sl[:], p1s[tc_i][:], silu_fn)
                    nc.vector.tensor_mul(
                        ht[:, tc_i * NCH : (tc_i + 1) * NCH], sl[:], p2s[tc_i][:]
                    )
                hts.append(ht)
                continue
            for tc_i in range(2):
                p1 = ps1.tile([128, NCH], f32, tag="p1")
                p2 = ps1.tile([128, NCH], f32, tag="p2")
                for k in range(KC):
                    nc.tensor.matmul(
                        p1[:],
                        lhsT=g[:, k, :],
                        rhs=xts[k // 2][:, (k % 2) * 2 + tc_i, :],
                        start=(k == 0),
                        stop=(k == KC - 1),
                    )
                for k in range(KC):
                    nc.tensor.matmul(
                        p2[:],
                        lhsT=g[:, KC + k, :],
                        rhs=xts[k // 2][:, (k % 2) * 2 + tc_i, :],
                        start=(k == 0),
                        stop=(k == KC - 1),
                    )
                sl = s_pool.tile([128, NCH], f32, tag="s")
                nc.scalar.activation(sl[:], p1[:], silu_fn)
                nc.vector.tensor_mul(ht[:, tc_i * NCH : (tc_i + 1) * NCH], sl[:], p2[:])
            hts.append(ht)

        # stage 2: out[m,hc] = sum_i hT_i[:, m].T @ down_i[:, hc]
        for hc in range(2):
            for m in range(TE // 128):
                po = ps3.tile([128, NCH], f32, tag="po")
                for i in range(IS):
                    nc.tensor.matmul(
                        po[:],
                        lhsT=hts[i][:, m * 128 : (m + 1) * 128],
                        rhs=dns[i // 2][:, (i % 2) * 2 + hc, :],
                        start=(i == 0),
                        stop=(i == IS - 1),
                    )
                ob = o_pool.tile([128, NCH], f32, tag="o")
                if hc == 1 and m == TE // 128 - 1:
                    # last tile: pipeline copy+DMA in halves to cut the
                    # exposed tail after the final matmul
                    nc.vector.tensor_copy(ob[:, 0:256], po[:, 0:256])
                    nc.scalar.dma_start(
                        out=out_d[m * 128 : (m + 1) * 128, hc * NCH : hc * NCH + 256],
                        in_=ob[:, 0:256],
                    )
                    nc.vector.tensor_copy(ob[:, 256:NCH], po[:, 256:NCH])
                    nc.sync.dma_start(
                        out=out_d[
                            m * 128 : (m + 1) * 128, hc * NCH + 256 : (hc + 1) * NCH
                        ],
                        in_=ob[:, 256:NCH],
                    )
                else:
                    nc.vector.tensor_copy(ob[:], po[:])
                    nc.scalar.dma_start(
                        out=out_d[m * 128 : (m + 1) * 128, hc * NCH : (hc + 1) * NCH],
                        in_=ob[:],
                    )

    nc.compile()
    return nc


def _get_nc():
    if "nc" not in _cache:
        _cache["nc"] = _build_nc()
    return _cache["nc"]


def _prep_in_maps(x, gate, up, down):
    """Slice per expert, pack/pair for wide contiguous device DMAs, cast bf16."""
    import ml_dtypes

    bf = ml_dtypes.bfloat16
    in_maps = []
    for e in range(E):
        # xt pairs: [NXP, 128p, q=(k%2)*2+tc, 512]
        a = x[e * TE : (e + 1) * TE].T.reshape(NXP, 2, 128, 2, NCH)
        xe = np.ascontiguousarray(a.transpose(0, 2, 1, 3, 4)).reshape(
            NXP, 128, 4, NCH
        )
        # gate/up [H, I] -> [IS, 128p(h%128), KC(h//128), 128m(i%128)], packed
        ge = gate[e].reshape(KC, 128, IS, 128).transpose(2, 1, 0, 3)
        ue = up[e].reshape(KC, 128, IS, 128).transpose(2, 1, 0, 3)
        gu = np.ascontiguousarray(np.concatenate([ge, ue], axis=2))
        # down pairs: [NDP, 128p, q=(i%2)*2+hc, 512]
        b = down[e].reshape(NDP, 2, 128, 2, NCH)
        de = np.ascontiguousarray(b.transpose(0, 2, 1, 3, 4)).reshape(
            NDP, 128, 4, NCH
        )
        in_maps.append(
            {
                "xt": xe.astype(bf),
                "gu": gu.astype(bf),
                "dn": de.astype(bf),
            }
        )
    return in_maps


def run(inputs: dict, trace: bool = False, tmpdir=None):
    """Full-input entry. Returns (output [T,H] f32, BassKernelResults|None)."""
    x = np.asarray(inputs["permuted_local_hidden_states"], dtype=np.float32)
    gate = np.asarray(inputs["grouped_gate_proj"], dtype=np.float32)
    up = np.asarray(inputs["grouped_up_proj"], dtype=np.float32)
    down = np.asarray(inputs["grouped_down_proj"], dtype=np.float32)
    tpe = np.asarray(inputs["tokens_per_expert"]).astype(np.int64)

    if not (x.shape == (T, H) and tpe.shape == (E,) and np.all(tpe == TE)):
        # general ragged fallback (host): correctness-only path
        out = np.empty((x.shape[0], down.shape[2]), dtype=np.float32)
        off = 0
        for e in range(E):
            n = int(tpe[e])
            xe = x[off : off + n]
            o1 = xe @ gate[e]
            o2 = xe @ up[e]
            with np.errstate(over="ignore"):
                hgl = (o1 / (1.0 + np.exp(-o1))) * o2
            out[off : off + n] = hgl @ down[e]
            off += n
        return out, None

    from concourse.bass_utils import run_bass_kernel_spmd

    nc = _get_nc()
    in_maps = _prep_in_maps(x, gate, up, down)
    res = run_bass_kernel_spmd(
        nc, in_maps, list(range(E)), trace=trace, tmpdir=tmpdir
    )
    out = np.concatenate([res.results[e]["out"] for e in range(E)], axis=0)
    return out, res


def kernel(**inputs) -> np.ndarray:
    out, _ = run(inputs, trace=False)
    return out


# revision 17
# speedup vs baseline: 1.0002x; 1.0002x over previous
"""Grouped MoE MLP (SwiGLU) for TRN2, expert-parallel across 8 NeuronCores.

Problem: T=8192 tokens pre-permuted into 8 contiguous expert segments of 1024,
H=1024, I=2816, per-expert weights gate/up [H,I], down [I,H].
    o1 = x @ gate; o2 = x @ up; h = silu(o1)*o2; out = h @ down

Sharding: expert-parallel - core e computes expert e's segment entirely
(zero collectives). Host slices inputs per expert and concatenates outputs.

Device kernel (per core), all matmul inputs bf16 (PSUM accumulation fp32,
rel err ~4e-3), N=512 moving dim (one PSUM bank):
  - stage 1: per I-slab of 128: o1T/o2T [128i, TE] = gate/up-slab.T @ xt,
    PSUM-accumulated over 8 H-chunks; SwiGLU fused: hT = silu(o1T)*o2T
    kept resident in SBUF ([I, TE] bf16, 22 slabs).
  - stage 2: out[TE, H] = hT.T @ down, PSUM-accumulated over 22 I-slabs.

Perf structure:
  - ~20 dependency-free warmup matmuls on scratch SBUF keep the PE busy
    from t~0 so the HAM clock-gate reaches 8/8 before real work and the
    DMA wait for the first tiles is hidden.
  - All DRAM layouts give every DMA >=2KB contiguous per-partition rows
    (4KB mostly): xt h-chunk pairs, gate+up packed per slab, down slab
    pairs. Each dma_start fans out across 16 HW queues.
  - dma_start issues cost ~0.6us of issuing-engine time each, so loads
    are spread round-robin over the Sync/GpSimd/Vector queues instead of
    serializing on one engine; outputs go on Scalar.
"""

import numpy as np
from contextlib import ExitStack

E, H, I, T = 8, 1024, 2816, 8192
TE = T // E  # tokens per expert = 1024
KC = H // 128  # 8 h-chunks
IS = I // 128  # 22 i-slabs
NCH = 512  # moving free dim per matmul (one PSUM bank of fp32)
NXP = KC // 2  # 4 xt pair tiles
NDP = IS // 2  # 11 down pair tiles
N_WARMUP = 22

_cache: dict = {}


def _build_nc():
    from concourse import bacc
    import concourse.tile as tile
    import concourse.mybir as mybir

    f32 = mybir.dt.float32
    bf = mybir.dt.bfloat16

    nc = bacc.Bacc("TRN2", target_bir_lowering=False, debug=False, num_devices=8)
    # xt pair j holds h-chunks k=2j,2j+1; quarter q = (k%2)*2 + tc
    xt_d = nc.dram_tensor("xt", [NXP, 128, 4, NCH], bf, kind="ExternalInput").ap()
    # gate+up packed per slab: slot k = gate h-chunk k, slot KC+k = up
    gu_d = nc.dram_tensor("gu", [IS, 128, 2 * KC, 128], bf, kind="ExternalInput").ap()
    # down pair j holds i-slabs 2j,2j+1; quarter q = (i%2)*2 + hc
    dn_d = nc.dram_tensor("dn", [NDP, 128, 4, NCH], bf, kind="ExternalInput").ap()
    out_d = nc.dram_tensor("out", [TE, H], f32, kind="ExternalOutput").ap()

    silu_fn = mybir.ActivationFunctionType.Silu

    with tile.TileContext(nc) as tc, ExitStack() as ctx:
        xt_pool = ctx.enter_context(tc.tile_pool(name="xt", bufs=NXP))
        gu_pool = ctx.enter_context(tc.tile_pool(name="gu", bufs=4))
        h_pool = ctx.enter_context(tc.tile_pool(name="h", bufs=IS))
        d_pool = ctx.enter_context(tc.tile_pool(name="d", bufs=NDP))
        s_pool = ctx.enter_context(tc.tile_pool(name="s", bufs=2))
        o_pool = ctx.enter_context(tc.tile_pool(name="o", bufs=2))
        w_pool = ctx.enter_context(tc.tile_pool(name="w", bufs=1))
        ps1 = ctx.enter_context(tc.tile_pool(name="ps1", bufs=2, space="PSUM"))
        ps3 = ctx.enter_context(tc.tile_pool(name="ps3", bufs=2, space="PSUM"))
        psw = ctx.enter_context(tc.tile_pool(name="psw", bufs=1, space="PSUM"))

        # PE warmup: dependency-free matmuls on scratch data so the PE is
        # busy while the first input DMAs are in flight (HAM un-throttles
        # after ~3.4us of sustained activity).
        dmy = w_pool.tile([128, 256], bf, tag="w", name="dmy")
        nc.vector.memset(dmy[:], 0)
        pw = psw.tile([128, 256], f32, tag="pw", name="pw")
        for _ in range(N_WARMUP):
            nc.tensor.matmul(
                pw[:], lhsT=dmy[:, 0:128], rhs=dmy[:], start=True, stop=True
            )

        # wave 1: critical input loads. Only Sync and Scalar have hardware
        # DGE paths (GpSimd DMAs fall back to slow software DGE - never
        # use them for bulk data). Transfers complete in ISSUE order (all
        # dma_starts share the same 16 HW queues FIFO at ~300GB/s), so the
        # order below is the arrival order, matched to slab-0's k-order
        # consumption; everything is split in 256KB halves so supply
        # granularity (~0.85us) matches the 4-matmuls-per-k consumption
        # rate (~0.86us).
        xts = [xt_pool.tile([128, 4, NCH], bf, tag="xt", name=f"xt{j}")
               for j in range(NXP)]
        gus = {}
        for i in range(3):
            gus[i] = gu_pool.tile([128, 2 * KC, 128], bf, tag="gu", name=f"gu{i}")
        nc.sync.dma_start(out=gus[0][:, 0:KC, :], in_=gu_d[0, :, 0:KC])
        nc.scalar.dma_start(out=gus[0][:, KC : 2 * KC, :], in_=gu_d[0, :, KC : 2 * KC])
        for j in range(NXP):
            nc.sync.dma_start(out=xts[j][:, 0:2, :], in_=xt_d[j, :, 0:2])
            nc.scalar.dma_start(out=xts[j][:, 2:4, :], in_=xt_d[j, :, 2:4])
        nc.sync.dma_start(out=gus[1][:, 0:KC, :], in_=gu_d[1, :, 0:KC])
        nc.scalar.dma_start(out=gus[1][:, KC : 2 * KC, :], in_=gu_d[1, :, KC : 2 * KC])
        nc.sync.dma_start(out=gus[2][:, 0:KC, :], in_=gu_d[2, :, 0:KC])
        nc.scalar.dma_start(out=gus[2][:, KC : 2 * KC, :], in_=gu_d[2, :, KC : 2 * KC])

        def issue(dst, src):
            nc.sync.dma_start(out=dst, in_=src)

        # stage 1: per i-slab, o1T/o2T then fused SwiGLU into resident hT
        hts = []
        dns = {}
        for i in range(IS):
            if i + 3 < IS:
                gus[i + 3] = gu_pool.tile(
                    [128, 2 * KC, 128], bf, tag="gu", name=f"gu{i + 3}"
                )
                issue(gus[i + 3][:], gu_d[i + 3])
            if 0 <= i - 10 < NDP:
                j = i - 10
                dns[j] = d_pool.tile([128, 4, NCH], bf, tag="d", name=f"dn{j}")
                issue(dns[j][:], dn_d[j])
            g = gus[i]
            ht = h_pool.tile([128, TE], bf, tag="h")
            if i == 0:
                # slab 0 is DMA-bound: interleave k across all four PSUM
                # groups so the PE consumes each arriving xt tile at the
                # DMA supply rate instead of idling for the full set.
                p1s = [ps1.tile([128, NCH], f32, tag="p1", name=f"p1_{t}")
                       for t in range(2)]
                p2s = [ps1.tile([128, NCH], f32, tag="p2", name=f"p2_{t}")
                       for t in range(2)]
                for k in range(KC):
                    for tc_i in range(2):
                        rhs = xts[k // 2][:, (k % 2) * 2 + tc_i, :]
                        nc.tensor.matmul(
                            p1s[tc_i][:], lhsT=g[:, k, :], rhs=rhs,
                            start=(k == 0), stop=(k == KC - 1),
                        )
                        nc.tensor.matmul(
                            p2s[tc_i][:], lhsT=g[:, KC + k, :], rhs=rhs,
                            start=(k == 0), stop=(k == KC - 1),
                        )
                for tc_i in range(2):
                    sl = s_pool.tile([128, NCH], f32, tag="s")
                    nc.scalar.activation(sl[:], p1s[tc_i][:], silu_fn)
                    nc.vector.tensor_mul(
                        ht[:, tc_i * NCH : (tc_i + 1) * NCH], sl[:], p2s[tc_i][:]
                    )
                hts.append(ht)
                continue
            for tc_i in range(2):
                p1 = ps1.tile([128, NCH], f32, tag="p1")
                p2 = ps1.tile([128, NCH], f32, tag="p2")
                for k in range(KC):
                    nc.tensor.matmul(
                        p1[:],
                        lhsT=g[:, k, :],
                        rhs=xts[k // 2][:, (k % 2) * 2 + tc_i, :],
                        start=(k == 0),
                        stop=(k == KC - 1),
                    )
                for k in range(KC):
                    nc.tensor.matmul(
                        p2[:],
                        lhsT=g[:, KC + k, :],
                        rhs=xts[k // 2][:, (k % 2) * 2 + tc_i, :],
                        start=(k == 0),
                        stop=(k == KC - 1),
                    )
                sl = s_pool.tile([128, NCH], f32, tag="s")
                nc.scalar.activation(sl[:], p1[:], silu_fn)
                nc.vector.tensor_mul(ht[:, tc_i * NCH : (tc_i + 1) * NCH], sl[:], p2[:])
            hts.append(ht)

        # stage 2: out[m,hc] = sum_i hT_i[:, m].T @ down_i[:, hc]
        for hc in range(2):
            for m in range(TE // 128):
                po = ps3.tile([128, NCH], f32, tag="po")
                for i in range(IS):
                    nc.tensor.matmul(
                        po[:],
                        lhsT=hts[i][:, m * 128 : (m + 1) * 128],
                        rhs=dns[i // 2][:, (i % 2) * 2 + hc, :],
                        start=(i == 0),
                        stop=(i == IS - 1),
                    )
                ob = o_pool.tile([128, NCH], f32, tag="o")
                if hc == 1 and m == TE // 128 - 1:
                    # last tile: pipeline copy+DMA in halves to cut the
                    # exposed tail after the final matmul
                    nc.vector.tensor_copy(ob[:, 0:256], po[:, 0:256])
                    nc.scalar.dma_start(
                        out=out_d[m * 128 : (m + 1) * 128, hc * NCH : hc * NCH + 256],
                        in_=ob[:, 0:256],
                    )
                    nc.vector.tensor_copy(ob[:, 256:NCH], po[:, 256:NCH])
                    nc.sync.dma_start(
                        out=out_d[
                            m * 128 : (m + 1) * 128, hc * NCH + 256 : (hc + 1) * NCH
                        ],
                        in_=ob[:, 256:NCH],
                    )
                else:
                    nc.vector.tensor_copy(ob[:], po[:])
                    nc.scalar.dma_start(
                        out=out_d[m * 128 : (m + 1) * 128, hc * NCH : (hc + 1) * NCH],
                        in_=ob[:],
                    )

    nc.compile()
    return nc


def _get_nc():
    if "nc" not in _cache:
        _cache["nc"] = _build_nc()
    return _cache["nc"]


def _prep_in_maps(x, gate, up, down):
    """Slice per expert, pack/pair for wide contiguous device DMAs, cast bf16."""
    import ml_dtypes

    bf = ml_dtypes.bfloat16
    in_maps = []
    for e in range(E):
        # xt pairs: [NXP, 128p, q=(k%2)*2+tc, 512]
        a = x[e * TE : (e + 1) * TE].T.reshape(NXP, 2, 128, 2, NCH)
        xe = np.ascontiguousarray(a.transpose(0, 2, 1, 3, 4)).reshape(
            NXP, 128, 4, NCH
        )
        # gate/up [H, I] -> [IS, 128p(h%128), KC(h//128), 128m(i%128)], packed
        ge = gate[e].reshape(KC, 128, IS, 128).transpose(2, 1, 0, 3)
        ue = up[e].reshape(KC, 128, IS, 128).transpose(2, 1, 0, 3)
        gu = np.ascontiguousarray(np.concatenate([ge, ue], axis=2))
        # down pairs: [NDP, 128p, q=(i%2)*2+hc, 512]
        b = down[e].reshape(NDP, 2, 128, 2, NCH)
        de = np.ascontiguousarray(b.transpose(0, 2, 1, 3, 4)).reshape(
            NDP, 128, 4, NCH
        )
        in_maps.append(
            {
                "xt": xe.astype(bf),
                "gu": gu.astype(bf),
                "dn": de.astype(bf),
            }
        )
    return in_maps


def run(inputs: dict, trace: bool = False, tmpdir=None):
    """Full-input entry. Returns (output [T,H] f32, BassKernelResults|None)."""
    x = np.asarray(inputs["permuted_local_hidden_states"], dtype=np.float32)
    gate = np.asarray(inputs["grouped_gate_proj"], dtype=np.float32)
    up = np.asarray(inputs["grouped_up_proj"], dtype=np.float32)
    down = np.asarray(inputs["grouped_down_proj"], dtype=np.float32)
    tpe = np.asarray(inputs["tokens_per_expert"]).astype(np.int64)

    if not (x.shape == (T, H) and tpe.shape == (E,) and np.all(tpe == TE)):
        # general ragged fallback (host): correctness-only path
        out = np.empty((x.shape[0], down.shape[2]), dtype=np.float32)
        off = 0
        for e in range(E):
            n = int(tpe[e])
            xe = x[off : off + n]
            o1 = xe @ gate[e]
            o2 = xe @ up[e]
            with np.errstate(over="ignore"):
                hgl = (o1 / (1.0 + np.exp(-o1))) * o2
            out[off : off + n] = hgl @ down[e]
            off += n
        return out, None

    from concourse.bass_utils import run_bass_kernel_spmd

    nc = _get_nc()
    in_maps = _prep_in_maps(x, gate, up, down)
    res = run_bass_kernel_spmd(
        nc, in_maps, list(range(E)), trace=trace, tmpdir=tmpdir
    )
    out = np.concatenate([res.results[e]["out"] for e in range(E)], axis=0)
    return out, res


def kernel(**inputs) -> np.ndarray:
    out, _ = run(inputs, trace=False)
    return out


# revision 18
# speedup vs baseline: 1.0004x; 1.0002x over previous
"""Grouped MoE MLP (SwiGLU) for TRN2, expert-parallel across 8 NeuronCores.

Problem: T=8192 tokens pre-permuted into 8 contiguous expert segments of 1024,
H=1024, I=2816, per-expert weights gate/up [H,I], down [I,H].
    o1 = x @ gate; o2 = x @ up; h = silu(o1)*o2; out = h @ down

Sharding: expert-parallel - core e computes expert e's segment entirely
(zero collectives). Host slices inputs per expert and concatenates outputs.

Device kernel (per core), all matmul inputs bf16 (PSUM accumulation fp32,
rel err ~4e-3), N=512 moving dim (one PSUM bank):
  - stage 1: per I-slab of 128: o1T/o2T [128i, TE] = gate/up-slab.T @ xt,
    PSUM-accumulated over 8 H-chunks; SwiGLU fused: hT = silu(o1T)*o2T
    kept resident in SBUF ([I, TE] bf16, 22 slabs).
  - stage 2: out[TE, H] = hT.T @ down, PSUM-accumulated over 22 I-slabs.

Perf structure:
  - ~20 dependency-free warmup matmuls on scratch SBUF keep the PE busy
    from t~0 so the HAM clock-gate reaches 8/8 before real work and the
    DMA wait for the first tiles is hidden.
  - All DRAM layouts give every DMA >=2KB contiguous per-partition rows
    (4KB mostly): xt h-chunk pairs, gate+up packed per slab, down slab
    pairs. Each dma_start fans out across 16 HW queues.
  - dma_start issues cost ~0.6us of issuing-engine time each, so loads
    are spread round-robin over the Sync/GpSimd/Vector queues instead of
    serializing on one engine; outputs go on Scalar.
"""

import numpy as np
from contextlib import ExitStack

E, H, I, T = 8, 1024, 2816, 8192
TE = T // E  # tokens per expert = 1024
KC = H // 128  # 8 h-chunks
IS = I // 128  # 22 i-slabs
NCH = 512  # moving free dim per matmul (one PSUM bank of fp32)
NXP = KC // 2  # 4 xt pair tiles
NDP = IS // 2  # 11 down pair tiles
N_WARMUP = 22

_cache: dict = {}


def _build_nc():
    from concourse import bacc
    import concourse.tile as tile
    import concourse.mybir as mybir

    f32 = mybir.dt.float32
    bf = mybir.dt.bfloat16

    nc = bacc.Bacc("TRN2", target_bir_lowering=False, debug=False, num_devices=8)
    # xt pair j holds h-chunks k=2j,2j+1; quarter q = (k%2)*2 + tc
    xt_d = nc.dram_tensor("xt", [NXP, 128, 4, NCH], bf, kind="ExternalInput").ap()
    # gate+up packed per slab: slot k = gate h-chunk k, slot KC+k = up
    gu_d = nc.dram_tensor("gu", [IS, 128, 2 * KC, 128], bf, kind="ExternalInput").ap()
    # down pair j holds i-slabs 2j,2j+1; quarter q = (i%2)*2 + hc
    dn_d = nc.dram_tensor("dn", [NDP, 128, 4, NCH], bf, kind="ExternalInput").ap()
    out_d = nc.dram_tensor("out", [TE, H], f32, kind="ExternalOutput").ap()

    silu_fn = mybir.ActivationFunctionType.Silu

    with tile.TileContext(nc) as tc, ExitStack() as ctx:
        xt_pool = ctx.enter_context(tc.tile_pool(name="xt", bufs=NXP))
        gu_pool = ctx.enter_context(tc.tile_pool(name="gu", bufs=4))
        h_pool = ctx.enter_context(tc.tile_pool(name="h", bufs=IS))
        d_pool = ctx.enter_context(tc.tile_pool(name="d", bufs=NDP))
        s_pool = ctx.enter_context(tc.tile_pool(name="s", bufs=2))
        o_pool = ctx.enter_context(tc.tile_pool(name="o", bufs=2))
        w_pool = ctx.enter_context(tc.tile_pool(name="w", bufs=1))
        ps1 = ctx.enter_context(tc.tile_pool(name="ps1", bufs=2, space="PSUM"))
        ps3 = ctx.enter_context(tc.tile_pool(name="ps3", bufs=2, space="PSUM"))
        psw = ctx.enter_context(tc.tile_pool(name="psw", bufs=1, space="PSUM"))

        # PE warmup: dependency-free matmuls on scratch data so the PE is
        # busy while the first input DMAs are in flight (HAM un-throttles
        # after ~3.4us of sustained activity).
        dmy = w_pool.tile([128, 256], bf, tag="w", name="dmy")
        nc.vector.memset(dmy[:], 0)
        pw = psw.tile([128, 256], f32, tag="pw", name="pw")
        for _ in range(N_WARMUP):
            nc.tensor.matmul(
                pw[:], lhsT=dmy[:, 0:128], rhs=dmy[:], start=True, stop=True
            )

        # wave 1: critical input loads. Only Sync and Scalar have hardware
        # DGE paths (GpSimd DMAs fall back to slow software DGE - never
        # use them for bulk data). Transfers complete in ISSUE order (all
        # dma_starts share the same 16 HW queues FIFO at ~300GB/s), so the
        # order below is the arrival order, matched to slab-0's k-order
        # consumption; everything is split in 256KB halves so supply
        # granularity (~0.85us) matches the 4-matmuls-per-k consumption
        # rate (~0.86us).
        xts = [xt_pool.tile([128, 4, NCH], bf, tag="xt", name=f"xt{j}")
               for j in range(NXP)]
        gus = {}
        for i in range(3):
            gus[i] = gu_pool.tile([128, 2 * KC, 128], bf, tag="gu", name=f"gu{i}")
        nc.sync.dma_start(out=gus[0][:, 0:KC, :], in_=gu_d[0, :, 0:KC])
        nc.scalar.dma_start(out=gus[0][:, KC : 2 * KC, :], in_=gu_d[0, :, KC : 2 * KC])
        for j in range(NXP):
            nc.sync.dma_start(out=xts[j][:, 0:2, :], in_=xt_d[j, :, 0:2])
            nc.scalar.dma_start(out=xts[j][:, 2:4, :], in_=xt_d[j, :, 2:4])
        nc.sync.dma_start(out=gus[1][:, 0:KC, :], in_=gu_d[1, :, 0:KC])
        nc.scalar.dma_start(out=gus[1][:, KC : 2 * KC, :], in_=gu_d[1, :, KC : 2 * KC])
        nc.sync.dma_start(out=gus[2][:, 0:KC, :], in_=gu_d[2, :, 0:KC])
        nc.scalar.dma_start(out=gus[2][:, KC : 2 * KC, :], in_=gu_d[2, :, KC : 2 * KC])

        def issue(dst, src):
            nc.sync.dma_start(out=dst, in_=src)

        # stage 1: per i-slab, o1T/o2T then fused SwiGLU into resident hT
        hts = []
        dns = {}
        for i in range(IS):
            if i + 3 < IS:
                gus[i + 3] = gu_pool.tile(
                    [128, 2 * KC, 128], bf, tag="gu", name=f"gu{i + 3}"
                )
                issue(gus[i + 3][:], gu_d[i + 3])
            if 0 <= i - 10 < NDP:
                j = i - 10
                dns[j] = d_pool.tile([128, 4, NCH], bf, tag="d", name=f"dn{j}")
                issue(dns[j][:], dn_d[j])
            g = gus[i]
            ht = h_pool.tile([128, TE], bf, tag="h")
            if i == 0:
                # slab 0 is DMA-bound: interleave k across all four PSUM
                # groups so the PE consumes each arriving xt tile at the
                # DMA supply rate instead of idling for the full set.
                p1s = [ps1.tile([128, NCH], f32, tag="p1", name=f"p1_{t}")
                       for t in range(2)]
                p2s = [ps1.tile([128, NCH], f32, tag="p2", name=f"p2_{t}")
                       for t in range(2)]
                for k in range(KC):
                    for tc_i in range(2):
                        rhs = xts[k // 2][:, (k % 2) * 2 + tc_i, :]
                        nc.tensor.matmul(
                            p1s[tc_i][:], lhsT=g[:, k, :], rhs=rhs,
                            start=(k == 0), stop=(k == KC - 1),
                        )
                        nc.tensor.matmul(
                            p2s[tc_i][:], lhsT=g[:, KC + k, :], rhs=rhs,
                            start=(k == 0), stop=(k == KC - 1),
                        )
                for tc_i in range(2):
                    sl = s_pool.tile([128, NCH], f32, tag="s")
                    nc.scalar.activation(sl[:], p1s[tc_i][:], silu_fn)
                    nc.vector.tensor_mul(
                        ht[:, tc_i * NCH : (tc_i + 1) * NCH], sl[:], p2s[tc_i][:]
                    )
                hts.append(ht)
                continue
            for tc_i in range(2):
                p1 = ps1.tile([128, NCH], f32, tag="p1")
                p2 = ps1.tile([128, NCH], f32, tag="p2")
                for k in range(KC):
                    nc.tensor.matmul(
                        p1[:],
                        lhsT=g[:, k, :],
                        rhs=xts[k // 2][:, (k % 2) * 2 + tc_i, :],
                        start=(k == 0),
                        stop=(k == KC - 1),
                    )
                for k in range(KC):
                    nc.tensor.matmul(
                        p2[:],
                        lhsT=g[:, KC + k, :],
                        rhs=xts[k // 2][:, (k % 2) * 2 + tc_i, :],
                        start=(k == 0),
                        stop=(k == KC - 1),
                    )
                sl = s_pool.tile([128, NCH], f32, tag="s")
                nc.scalar.activation(sl[:], p1[:], silu_fn)
                nc.vector.tensor_mul(ht[:, tc_i * NCH : (tc_i + 1) * NCH], sl[:], p2[:])
            hts.append(ht)

        # stage 2: out[m,hc] = sum_i hT_i[:, m].T @ down_i[:, hc]
        for hc in range(2):
            for m in range(TE // 128):
                last = hc == 1 and m == TE // 128 - 1
                if last:
                    # final tile: two half-width PSUM groups so the first
                    # half's copy+DMA completes under the second half's
                    # matmuls, halving the exposed tail after the last MM
                    ob = o_pool.tile([128, NCH], f32, tag="o")
                    for half, eng in ((0, nc.scalar), (1, nc.sync)):
                        ph = ps3.tile([128, NCH // 2], f32, tag="po",
                                      name=f"po_h{half}")
                        for i in range(IS):
                            nc.tensor.matmul(
                                ph[:],
                                lhsT=hts[i][:, m * 128 : (m + 1) * 128],
                                rhs=dns[i // 2][
                                    :, (i % 2) * 2 + hc,
                                    half * 256 : (half + 1) * 256,
                                ],
                                start=(i == 0),
                                stop=(i == IS - 1),
                            )
                        cs = slice(half * 256, (half + 1) * 256)
                        nc.vector.tensor_copy(ob[:, cs], ph[:])
                        eng.dma_start(
                            out=out_d[
                                m * 128 : (m + 1) * 128,
                                hc * NCH + half * 256 : hc * NCH + (half + 1) * 256,
                            ],
                            in_=ob[:, cs],
                        )
                    continue
                po = ps3.tile([128, NCH], f32, tag="po")
                for i in range(IS):
                    nc.tensor.matmul(
                        po[:],
                        lhsT=hts[i][:, m * 128 : (m + 1) * 128],
                        rhs=dns[i // 2][:, (i % 2) * 2 + hc, :],
                        start=(i == 0),
                        stop=(i == IS - 1),
                    )
                ob = o_pool.tile([128, NCH], f32, tag="o")
                nc.vector.tensor_copy(ob[:], po[:])
                nc.scalar.dma_start(
                    out=out_d[m * 128 : (m + 1) * 128, hc * NCH : (hc + 1) * NCH],
                    in_=ob[:],
                )

    nc.compile()
    return nc


def _get_nc():
    if "nc" not in _cache:
        _cache["nc"] = _build_nc()
    return _cache["nc"]


def _prep_in_maps(x, gate, up, down):
    """Slice per expert, pack/pair for wide contiguous device DMAs, cast bf16."""
    import ml_dtypes

    bf = ml_dtypes.bfloat16
    in_maps = []
    for e in range(E):
        # xt pairs: [NXP, 128p, q=(k%2)*2+tc, 512]
        a = x[e * TE : (e + 1) * TE].T.reshape(NXP, 2, 128, 2, NCH)
        xe = np.ascontiguousarray(a.transpose(0, 2, 1, 3, 4)).reshape(
            NXP, 128, 4, NCH
        )
        # gate/up [H, I] -> [IS, 128p(h%128), KC(h//128), 128m(i%128)], packed
        ge = gate[e].reshape(KC, 128, IS, 128).transpose(2, 1, 0, 3)
        ue = up[e].reshape(KC, 128, IS, 128).transpose(2, 1, 0, 3)
        gu = np.ascontiguousarray(np.concatenate([ge, ue], axis=2))
        # down pairs: [NDP, 128p, q=(i%2)*2+hc, 512]
        b = down[e].reshape(NDP, 2, 128, 2, NCH)
        de = np.ascontiguousarray(b.transpose(0, 2, 1, 3, 4)).reshape(
            NDP, 128, 4, NCH
        )
        in_maps.append(
            {
                "xt": xe.astype(bf),
                "gu": gu.astype(bf),
                "dn": de.astype(bf),
            }
        )
    return in_maps


def run(inputs: dict, trace: bool = False, tmpdir=None):
    """Full-input entry. Returns (output [T,H] f32, BassKernelResults|None)."""
    x = np.asarray(inputs["permuted_local_hidden_states"], dtype=np.float32)
    gate = np.asarray(inputs["grouped_gate_proj"], dtype=np.float32)
    up = np.asarray(inputs["grouped_up_proj"], dtype=np.float32)
    down = np.asarray(inputs["grouped_down_proj"], dtype=np.float32)
    tpe = np.asarray(inputs["tokens_per_expert"]).astype(np.int64)

    if not (x.shape == (T, H) and tpe.shape == (E,) and np.all(tpe == TE)):
        # general ragged fallback (host): correctness-only path
        out = np.empty((x.shape[0], down.shape[2]), dtype=np.float32)
        off = 0
        for e in range(E):
            n = int(tpe[e])
            xe = x[off : off + n]
            o1 = xe @ gate[e]
            o2 = xe @ up[e]
            with np.errstate(over="ignore"):
                hgl = (o1 / (1.0 + np.exp(-o1))) * o2
            out[off : off + n] = hgl @ down[e]
            off += n
        return out, None

    from concourse.bass_utils import run_bass_kernel_spmd

    nc = _get_nc()
    in_maps = _prep_in_maps(x, gate, up, down)
    res = run_bass_kernel_spmd(
        nc, in_maps, list(range(E)), trace=trace, tmpdir=tmpdir
    )
    out = np.concatenate([res.results[e]["out"] for e in range(E)], axis=0)
    return out, res


def kernel(**inputs) -> np.ndarray:
    out, _ = run(inputs, trace=False)
    return out
